# revision 12
# baseline (speedup 1.0000x reference)
"""Trainium2 Bass kernel for nn_Decoder_77051713290327.

2-layer stacked LSTM (shared Keras LSTMCell weights W/U/b, gate order
i|f|g|o), B=256, T=1024, F=UNITS=128, fp32, return_sequences=False.
reference() returns (h2_last[256,128], zeros[256,1024,128], c1_last[256,128]).

Strategy: data-parallel over batch across 8 NeuronCores (32 batch each),
small shared weights replicated.  Per core, a latency-optimized fused
recurrence:

- Layout: units on SBUF partitions, batch on the free dim; layer1 and the
  (lag-2 pipelined) layer2 stacked side by side in every tile.
- The per-core batch is split into n_chains independent recurrences that
  interleave on the engines: while one chain runs its activations, another
  runs its matmuls, hiding the serial cross-engine latency of each chain.
- Matmuls: fp16 operands (weights stationary [128,128] per gate, moving rhs
  [128,2*bc]), fp32 PSUM accumulate.  Per chain-iteration 8 matmuls:
  z1(t) = x_t@W + h1_{t-1}@U alongside z2(t-2) = h1_{t-2}@W + h2_{t-3}@U,
  sharing weight loads and rhs streams via column stacking.
- Activations: host prescales weights so ONE tanh(0.5*z) ScalarE instruction
  yields tanh for the g gate and (2*sigmoid-1) for i,f,o; states are kept
  doubled (C=2c, H=2h) so the cell update is 4 fused scalar_tensor_tensor
  VectorE ops + 1 tanh.
- An SBUF ring of fp16 rhs slots [x | H1 | H2] lets the hidden-state write
  of one iteration land directly inside later iterations' matmul rhs.
"""

import numpy as np

import concourse.bass as bass
import concourse.bacc as bacc
import concourse.tile as tile
from concourse import mybir
from concourse.bass_utils import run_bass_kernel_spmd

P = 128          # units / partitions
BC = 32          # batch per core
NCORES = 8
NG = 4           # gates
FP32 = mybir.dt.float32

ADD = mybir.AluOpType.add
MULT = mybir.AluOpType.mult
TANH = mybir.ActivationFunctionType.Tanh


def _build(T, U, with_bias, n_chains=2, mm_dtype=mybir.dt.float16):
    assert T % U == 0 and U % 2 == 0 and U >= 4
    R = U
    f16 = mm_dtype
    bc = BC // n_chains          # batch per chain
    SLOT = 3 * bc                # ring slot stride (x | H1 | H2)
    nc = bacc.Bacc("TRN2")

    # x input: [T+2, chain, P, bc] so each chain's per-step slice is
    # partition-contiguous; rows T, T+1 are zeros (epilogue drain).
    xin = nc.dram_tensor("xin", [T + 2, n_chains, P, bc], f16, kind="ExternalInput")
    wd = nc.dram_tensor("wd", [P, NG * P], f16, kind="ExternalInput")
    ud = nc.dram_tensor("ud", [P, NG * P], f16, kind="ExternalInput")
    if with_bias:
        bd = nc.dram_tensor("bd", [P, NG], FP32, kind="ExternalInput")
    h2o = nc.dram_tensor("h2o", [P, BC], FP32, kind="ExternalOutput")
    c1o = nc.dram_tensor("c1o", [P, BC], FP32, kind="ExternalOutput")

    with tile.TileContext(nc) as tc:
        with (
            tc.tile_pool(name="const", bufs=1) as constp,
            tc.tile_pool(name="state", bufs=1) as statep,
            tc.tile_pool(name="work", bufs=3) as workp,
            tc.tile_pool(name="psum", bufs=1, space="PSUM") as psump,
        ):
            w_sb = constp.tile([P, NG * P], f16, tag="w")
            u_sb = constp.tile([P, NG * P], f16, tag="u")
            nc.sync.dma_start(w_sb[:], wd[:])
            nc.sync.dma_start(u_sb[:], ud[:])
            b_sb = None
            if with_bias:
                b_sb = constp.tile([P, NG], FP32, tag="bias")
                nc.sync.dma_start(b_sb[:], bd[:])

            chains = []
            for k in range(n_chains):
                ch = {
                    "ring": statep.tile([P, R * SLOT], f16, tag=f"ring{k}",
                                        name=f"ring{k}"),
                    "cst": statep.tile([P, 4 * bc], FP32, tag=f"cst{k}",
                                       name=f"cst{k}"),
                    # one persistent PSUM tile per chain: gate g in bank g
                    "Z": psump.tile([P, NG * 512], FP32, tag=f"z{k}",
                                    name=f"zps{k}"),
                }
                nc.vector.memset(ch["ring"][:], 0.0)
                nc.vector.memset(ch["cst"][:], 0.0)
                chains.append(ch)
            outsb = statep.tile([P, 2 * BC], FP32, tag="outsb")

            def stage_mm(k, s, tau):
                ch = chains[k]
                ring, Z = ch["ring"], ch["Z"]
                s1 = (s + 1) % R
                nc.sync.dma_start(ring[:, s * SLOT : s * SLOT + bc], xin[tau, k])
                wrhs = ring[:, s * SLOT : s * SLOT + 2 * bc]
                urhs = ring[:, s1 * SLOT + bc : s1 * SLOT + 3 * bc]
                for g in range(NG):
                    nc.tensor.matmul(
                        Z[:, g * 512 : g * 512 + 2 * bc],
                        w_sb[:, g * P : (g + 1) * P],
                        wrhs, start=True, stop=False,
                    )
                for g in range(NG):
                    nc.tensor.matmul(
                        Z[:, g * 512 : g * 512 + 2 * bc],
                        u_sb[:, g * P : (g + 1) * P],
                        urhs, start=False, stop=True,
                    )

            def stage_gates(k, work):
                ch = chains[k]
                Z = ch["Z"]
                Tt = workp.tile([P, NG * 2 * bc], FP32, tag=f"tt{k}", name=f"tt{k}")
                work["Tt"] = Tt
                Zg = Z[:].rearrange("p (g n) -> p g n", g=NG)[:, :, 0 : 2 * bc]
                Ttg = Tt[:].rearrange("p (g n) -> p g n", g=NG)
                if with_bias:
                    for g in range(NG):
                        nc.scalar.activation(
                            Tt[:, g * 2 * bc : (g + 1) * 2 * bc],
                            Z[:, g * 512 : g * 512 + 2 * bc],
                            TANH, bias=b_sb[:, g : g + 1], scale=0.5,
                        )
                else:
                    nc.scalar.activation(Ttg, Zg, TANH, scale=0.5)

            def stage_cell(k, s, work):
                ch = chains[k]
                cst = ch["cst"]
                Tt = work["Tt"]
                par = s % 2
                parp = (par + 1) % 2
                Ti = Tt[:, 0 * 2 * bc : 1 * 2 * bc]
                Tf = Tt[:, 1 * 2 * bc : 2 * 2 * bc]
                Tg = Tt[:, 2 * 2 * bc : 3 * 2 * bc]
                Cprev = cst[:, parp * 2 * bc : (parp + 1) * 2 * bc]
                Ccur = cst[:, par * 2 * bc : (par + 1) * 2 * bc]
                A = workp.tile([P, 2 * bc], FP32, tag=f"a{k}", name=f"a{k}")
                Bt = workp.tile([P, 2 * bc], FP32, tag=f"b{k}", name=f"b{k}")
                work["Ccur"] = Ccur
                nc.vector.scalar_tensor_tensor(A[:], Ti, 1.0, Tg, op0=ADD, op1=MULT)
                nc.vector.scalar_tensor_tensor(Bt[:], Tf, 1.0, Cprev, op0=ADD, op1=MULT)
                nc.vector.scalar_tensor_tensor(Ccur, Bt[:], 0.5, A[:], op0=MULT, op1=ADD)

            def stage_tanhc(k, work):
                Tc = workp.tile([P, 2 * bc], FP32, tag=f"tc{k}", name=f"tc{k}")
                work["Tc"] = Tc
                nc.scalar.activation(Tc[:], work["Ccur"], TANH, scale=0.5)

            def stage_h(k, s, work):
                ch = chains[k]
                ring = ch["ring"]
                s2 = (s + 2) % R
                Tt, Tc = work["Tt"], work["Tc"]
                To = Tt[:, 3 * 2 * bc : 4 * 2 * bc]
                nc.vector.scalar_tensor_tensor(
                    ring[:, s2 * SLOT + bc : s2 * SLOT + 3 * bc],
                    To, 1.0, Tc[:], op0=ADD, op1=MULT,
                )

            def iteration(k, s, tau):
                work = {}
                stage_mm(k, s, tau)
                stage_gates(k, work)
                stage_cell(k, s, work)
                stage_tanhc(k, work)
                stage_h(k, s, work)

            for tau in range(T):
                s = tau % R
                works = [{} for _ in range(n_chains)]
                for k in range(n_chains):
                    stage_mm(k, s, tau)
                for k in range(n_chains):
                    stage_gates(k, works[k])
                for k in range(n_chains):
                    stage_cell(k, s, works[k])
                for k in range(n_chains):
                    stage_tanhc(k, works[k])
                for k in range(n_chains):
                    stage_h(k, s, works[k])

            # c1 final = 0.5 * C(tau=T-1)[c1-half]; parity (T-1)%2
            par_last = (T - 1) % 2
            for k in range(n_chains):
                nc.vector.tensor_scalar_mul(
                    outsb[:, k * bc : (k + 1) * bc],
                    chains[k]["cst"][:, par_last * 2 * bc : par_last * 2 * bc + bc],
                    0.5,
                )
            nc.sync.dma_start(c1o[:], outsb[:, 0:BC])

            # epilogue: 2 drain iterations (x rows T, T+1 are zeros)
            for tau in (T, T + 1):
                for k in range(n_chains):
                    iteration(k, tau % R, tau)

            # h2 final = 0.5 * H2[T-1]  (ring slot (T+1+2)%R, cols 2bc:3bc)
            s_fin = (T + 3) % R
            for k in range(n_chains):
                nc.vector.tensor_scalar_mul(
                    outsb[:, BC + k * bc : BC + (k + 1) * bc],
                    chains[k]["ring"][:, s_fin * SLOT + 2 * bc : s_fin * SLOT + 3 * bc],
                    0.5,
                )
            nc.sync.dma_start(h2o[:], outsb[:, BC : 2 * BC])

    nc.finalize()
    return nc


def _prep_core_inputs(x_core, W_eff, U_eff, beff, T, n_chains):
    """x_core: [BC, T, P] fp32 -> kernel input dict for one core."""
    bc = BC // n_chains
    x2 = (2.0 * x_core.transpose(1, 2, 0)).astype(np.float16)  # [T, P, BC]
    xin = np.zeros((T + 2, n_chains, P, bc), np.float16)
    for k in range(n_chains):
        xin[:T, k] = x2[:, :, k * bc : (k + 1) * bc]
    out = {"xin": np.ascontiguousarray(xin), "wd": W_eff, "ud": U_eff}
    if beff is not None:
        out["bd"] = beff
    return out


def _run(inputs, W, U, b, T, Uu, n_chains=2, trace=False, tmpdir=None):
    Bfull = inputs.shape[0]
    assert Bfull == NCORES * BC
    with_bias = bool(np.any(np.asarray(b) != 0))

    gscale = np.ones((NG * P,), np.float32)
    gscale[2 * P : 3 * P] = 2.0
    W_eff = (np.asarray(W) * gscale[None, :] * 0.5).astype(np.float16)
    U_eff = (np.asarray(U) * gscale[None, :] * 0.5).astype(np.float16)
    beff = None
    if with_bias:
        b = np.asarray(b, dtype=np.float32)
        beff = np.empty((P, NG), np.float32)
        beff[:, 0] = 0.5 * b[0 * P : 1 * P]
        beff[:, 1] = 0.5 * b[1 * P : 2 * P]
        beff[:, 2] = 1.0 * b[2 * P : 3 * P]
        beff[:, 3] = 0.5 * b[3 * P : 4 * P]

    nc = _build(T, Uu, with_bias, n_chains=n_chains)
    x = np.asarray(inputs, dtype=np.float32)
    in_maps = [
        _prep_core_inputs(x[c * BC : (c + 1) * BC], W_eff, U_eff, beff, T, n_chains)
        for c in range(NCORES)
    ]
    res = run_bass_kernel_spmd(
        nc, in_maps, core_ids=list(range(NCORES)), trace=trace, tmpdir=tmpdir
    )
    h2 = np.empty((Bfull, P), np.float32)
    c1 = np.empty((Bfull, P), np.float32)
    for c in range(NCORES):
        h2[c * BC : (c + 1) * BC] = res.results[c]["h2o"].T
        c1[c * BC : (c + 1) * BC] = res.results[c]["c1o"].T
    return h2, c1, res


def _kernel_numpy(inputs, W, U, b):
    """Plain numpy fallback (unexpected shapes only)."""
    B, T, F = inputs.shape
    un = U.shape[0]
    h1 = np.zeros((B, un), np.float32); c1 = np.zeros((B, un), np.float32)
    h2 = np.zeros((B, un), np.float32); c2 = np.zeros((B, un), np.float32)
    sig = lambda v: 1.0 / (1.0 + np.exp(-v))

    def cell(xt, h, c):
        z = xt @ W + h @ U + b
        i, f, g, o = np.split(z, 4, axis=-1)
        cn = sig(f) * c + sig(i) * np.tanh(g)
        return (sig(o) * np.tanh(cn)).astype(np.float32), cn.astype(np.float32)

    for t in range(T):
        h1, c1 = cell(inputs[:, t], h1, c1)
        h2, c2 = cell(h1, h2, c2)
    return h2, c1


def kernel(inputs, W, U, b):
    inputs = np.asarray(inputs, dtype=np.float32)
    W = np.asarray(W, dtype=np.float32)
    U = np.asarray(U, dtype=np.float32)
    b = np.asarray(b, dtype=np.float32)
    B, T, F = inputs.shape
    if B != NCORES * BC or F != P or W.shape != (P, NG * P):
        h2, c1 = _kernel_numpy(inputs, W, U, b)
    else:
        h2, c1, _ = _run(inputs, W, U, b, T, 16)
    init_state = np.zeros(inputs.shape, dtype=np.float32)
    return h2, init_state, c1
